# revision 7
# baseline (speedup 1.0000x reference)
"""GEMV kernel for Trainium2: out = x @ W.T + b, sharded over 8 NeuronCores.

Shapes (hardcoded): x [1, 147456] f32, W [1000, 147456] f32, b [1000] f32.
Sharding: W's 1000 output classes split 8 ways (125 rows/core); each core
computes its 125 dot products locally, outputs concatenate host-side.

Per-core strategy (memory-bound: 73.7 MB of W per core at ~360 GB/s):
  - View k = k2*1152 + k1, so each W row [147456] maps to an SBUF tile
    [128 (k2, partitions), 1152 (k1, free)] with 4.6 KB contiguous per
    partition -> DMA at line rate.
  - x is reshaped the same way and stays resident in SBUF.
  - One fused DVE scalar_tensor_tensor per row m: out = (W*1.0)*x with
    accum_out = per-partition sum, i.e. multiply + free-dim reduce in a
    single pass (~1.05 cycles per element-per-partition), accumulating
    per-k2 partial sums into column m of a [128, 125] accumulator.
    (tensor_tensor_reduce would be the natural op but its opcode crashes
    the exec unit on this HW/compiler; TensorScalarPtr works.)
  - A single PE matmul against a ones vector reduces the accumulator across
    partitions -> [1, 125]; add bias; DMA out.
"""

import numpy as np

import concourse.bacc as bacc
import concourse.mybir as mybir
import concourse.tile as tile
from concourse.bass_utils import run_bass_kernel_spmd

N_CORES = 8
N_CLASSES = 1000
N_IN = 147456
P = 128                      # partitions (k2)
K1 = N_IN // P               # 1152 free elements per partition
M = N_CLASSES // N_CORES     # 125 rows per core
MT = 5                       # W rows per DMA chunk (25 chunks of 2.95 MB)

_prog_cache = {}


def _build_program():
    if "nc" in _prog_cache:
        return _prog_cache["nc"]

    nc = bacc.Bacc("TRN2", target_bir_lowering=False, debug=False, num_devices=N_CORES)
    f32 = mybir.dt.float32
    x_d = nc.dram_tensor("x", [P, K1], f32, kind="ExternalInput")
    w_d = nc.dram_tensor("W", [M, P, K1], f32, kind="ExternalInput")
    b_d = nc.dram_tensor("b", [1, M], f32, kind="ExternalInput")
    o_d = nc.dram_tensor("out", [1, M], f32, kind="ExternalOutput")
    ones_d = nc.inline_tensor(np.ones((P, 1), np.float32), "ones_const")

    with tile.TileContext(nc) as tc:
        with (
            tc.tile_pool(name="xpool", bufs=1) as xpool,
            tc.tile_pool(name="wpool", bufs=3) as wpool,
            tc.tile_pool(name="misc", bufs=1) as misc,
            tc.tile_pool(name="psum", bufs=1, space="PSUM") as psum_pool,
        ):
            x_t = xpool.tile([P, K1], f32)
            nc.sync.dma_start(x_t[:], x_d[:])
            ones_t = misc.tile([P, 1], f32)
            nc.sync.dma_start(ones_t[:], ones_d[:])
            b_t = misc.tile([1, M], f32)
            nc.sync.dma_start(b_t[:], b_d[:])
            acc_t = misc.tile([P, M], f32)

            for c in range(0, M, MT):
                mt = min(MT, M - c)
                w_t = wpool.tile([P, MT, K1], f32, tag="w")
                nc.sync.dma_start(
                    w_t[:, :mt, :], w_d[c : c + mt].rearrange("m p k -> p m k")
                )
                for j in range(mt):
                    m = c + j
                    dummy_t = wpool.tile([P, K1], f32, tag="s")
                    nc.vector.scalar_tensor_tensor(
                        out=dummy_t[:],
                        in0=w_t[:, j, :],
                        scalar=1.0,
                        in1=x_t[:],
                        op0=mybir.AluOpType.mult,
                        op1=mybir.AluOpType.mult,
                        accum_out=acc_t[:, m : m + 1],
                    )

            ps = psum_pool.tile([1, M], f32)
            nc.tensor.matmul(ps[:], ones_t[:], acc_t[:], start=True, stop=True)
            out_t = misc.tile([1, M], f32)
            nc.vector.tensor_add(out_t[:], ps[:], b_t[:])
            nc.sync.dma_start(o_d[:], out_t[:])

    nc.finalize()
    _prog_cache["nc"] = nc
    return nc


def _in_maps(x, W, b):
    x128 = np.ascontiguousarray(np.asarray(x).reshape(P, K1), dtype=np.float32)
    in_maps = []
    for c in range(N_CORES):
        sl = slice(c * M, (c + 1) * M)
        in_maps.append(
            {
                "x": x128,
                "W": np.ascontiguousarray(W[sl].reshape(M, P, K1), dtype=np.float32),
                "b": np.ascontiguousarray(b[sl].reshape(1, M), dtype=np.float32),
            }
        )
    return in_maps


def _run(x, W, b, trace=False, **kwargs):
    nc = _build_program()
    in_maps = _in_maps(x, W, b)
    return run_bass_kernel_spmd(nc, in_maps, list(range(N_CORES)), trace=trace, **kwargs)


def kernel(x, W, b):
    res = _run(x, W, b)
    outs = [r["out"].reshape(1, M) for r in res.results]
    return np.concatenate(outs, axis=1).astype(np.float32)
